# revision 1
# baseline (speedup 1.0000x reference)
"""Conv2d(128->256, 3x3, pad=1) over (32,128,56,56), data-parallel across 8
NeuronCores (4 images per core).

Per core: conv = 9 shifted accumulating matmuls per output tile.
  - contraction K = Cin = 128 (partition dim)
  - stationary lhsT = W^T[ci, co_tile] per (ky,kx)  -> [128, 128] bf16
  - moving rhs = input pixels [128, <=8 rows, <=56 cols] (N <= 448)
  - PSUM accumulates the 9 (ky,kx) taps; padding handled by clipping each
    tap's matmul to the valid rectangle (center tap goes first with
    start=True and covers the full tile, so partial-range taps accumulate
    on top via PSUM's per-element has_written bits).
Bias is added during the PSUM->SBUF copy (VectorE tensor_scalar).

Latency structure:
  - first image is loaded in row-quarters on the Scalar HWDGE ring while
    the weights load in cot-halves on the Sync HWDGE ring, so the first
    matmul can start as soon as quarter 0 + weight half 0 land;
  - a handful of zero dummy matmuls bridge the PE from the preamble to the
    first data-dependent matmul so the HAM clock-gate warms early;
  - images 1..3 prefetch on the GpSimd SWDGE queue;
  - output stores go out in row-quarters alternating Sync/Scalar rings so
    the final store before the exit barrier is small.
"""

import numpy as np
import ml_dtypes

import concourse.mybir as mybir
import concourse.tile as tile
from concourse import bacc
from concourse.bass_utils import run_bass_kernel_spmd

N_CORES = 8
B, CIN, H, W = 32, 128, 56, 56
COUT, R, S = 256, 3, 3
BL = B // N_CORES          # images per core
NCOT = COUT // 128         # Cout tiles of 128
YCHUNK = 8                 # output rows per matmul tile
NYC = H // YCHUNK

MM_DT = mybir.dt.bfloat16
MM_NP = ml_dtypes.bfloat16

NWARM = 6                  # dummy matmuls to bridge PE from preamble to data
X0_SPLITS = [0, 16, 32, 44, 56]       # first-image load quarters (rows)
OUT_SPLITS = {1: (0, 14), 3: (14, 28), 5: (28, 48), 6: (48, 56)}  # yc -> store rows
# tap order in the weight layout: center tap first (it is the start=True
# matmul that covers the full PSUM tile)
TAP_ORDER = [(1, 1), (0, 0), (0, 1), (0, 2), (1, 0), (1, 2), (2, 0), (2, 1), (2, 2)]

_cache = {}


def _build():
    if "nc" in _cache:
        return _cache["nc"]
    nc = bacc.Bacc("TRN2", target_bir_lowering=False, debug=False)
    f32 = mybir.dt.float32
    x_d = nc.dram_tensor("x", [BL, CIN, H, W], MM_DT, kind="ExternalInput").ap()
    w_d = nc.dram_tensor("w", [CIN, NCOT, R * S, 128], MM_DT, kind="ExternalInput").ap()
    b_d = nc.dram_tensor("b", [128, NCOT], f32, kind="ExternalInput").ap()
    y_d = nc.dram_tensor("y", [BL, COUT, H, W], f32, kind="ExternalOutput").ap()

    with tile.TileContext(nc) as tc:
        with (
            tc.tile_pool(name="consts", bufs=1) as cpool,
            tc.tile_pool(name="xin", bufs=BL) as xpool,
            tc.tile_pool(name="yout", bufs=2) as opool,
            tc.tile_pool(name="ps", bufs=8, space="PSUM") as pspool,
        ):
            # --- PE prewarm: zero matmuls with no DMA dependency ---
            warm_x = cpool.tile([128, 512], MM_DT)
            nc.vector.memset(warm_x[:], 0.0)
            warm_ps = pspool.tile([128, 512], f32, tag="ps")
            for _ in range(NWARM):
                nc.tensor.matmul(
                    warm_ps[:], warm_x[:, 0:128], warm_x[:], start=True, stop=True
                )

            # --- constants + first image, on parallel HWDGE rings ---
            # one DMA per cot half: per-DMA fixed latency dominates at these
            # sizes, so finer splits arrive LATER (measured)
            w_sb = cpool.tile([CIN, NCOT, R * S, 128], MM_DT)
            # cot-0 taps split across two rings so both halves land in
            # parallel (first DMA per ring pays the same ~4.5us e2e latency)
            nc.sync.dma_start(w_sb[:, 0, 0:5], w_d[:, 0, 0:5])
            nc.gpsimd.dma_start(w_sb[:, 0, 5:], w_d[:, 0, 5:])
            nc.sync.dma_start(w_sb[:, 1], w_d[:, 1])
            b_sb = cpool.tile([128, NCOT], f32)
            nc.sync.dma_start(b_sb[:], b_d[:])

            # all images have dedicated SBUF slots (bufs=BL), so every load
            # issues up-front with no slot-wait stalling the issuing engine;
            # GpSimd does no DMA at all (no SWDGE queue in play)
            x_tiles = []
            x0 = xpool.tile([CIN, H, W], MM_DT, name="x_sb_0", tag="x_sb")
            for r0, r1 in zip(X0_SPLITS, X0_SPLITS[1:]):
                nc.scalar.dma_start(x0[:, r0:r1, :], x_d[0, :, r0:r1, :])
            x_tiles.append(x0)
            for img in range(1, BL):
                x_sb = xpool.tile([CIN, H, W], MM_DT, name=f"x_sb_{img}", tag="x_sb")
                nc.sync.dma_start(x_sb[:], x_d[img])
                x_tiles.append(x_sb)

            for img in range(BL):
                x_sb = x_tiles[img]
                for cot in range(NCOT):
                    o_sb = opool.tile(
                        [128, H, W], f32, name=f"o_sb_{img}_{cot}", tag="o_sb"
                    )
                    for yc in range(NYC):
                        y0 = YCHUNK * yc
                        ps = pspool.tile(
                            [128, YCHUNK, W], f32, name=f"ps_{img}_{cot}_{yc}", tag="ps"
                        )
                        # center tap first: full-tile write with start=True
                        nc.tensor.matmul(
                            ps[:],
                            w_sb[:, cot, 0, :],
                            x_sb[:, y0 : y0 + YCHUNK, :],
                            start=True,
                            stop=False,
                        )
                        for ti, (ky, kx) in enumerate(TAP_ORDER[1:], start=1):
                            oy0 = max(0, 1 - ky - y0)
                            oy1 = min(YCHUNK, H + 1 - y0 - ky)
                            ox0 = max(0, 1 - kx)
                            ox1 = min(W, W + 1 - kx)
                            nc.tensor.matmul(
                                ps[:, oy0:oy1, ox0:ox1],
                                w_sb[:, cot, ti, :],
                                x_sb[
                                    :,
                                    y0 + oy0 + ky - 1 : y0 + oy1 + ky - 1,
                                    ox0 + kx - 1 : ox1 + kx - 1,
                                ],
                                start=False,
                                stop=(ti == R * S - 1),
                            )
                        # PSUM -> SBUF with fused bias add, all on VectorE
                        # (no ACTIVATE => Scalar never loads its LUT and is a
                        # pure DMA-issue engine)
                        nc.vector.tensor_scalar_add(
                            o_sb[:, y0 : y0 + YCHUNK, :],
                            ps[:],
                            b_sb[:, cot : cot + 1],
                        )
                        # store finished row-quarters, alternating HWDGE rings
                        if yc in OUT_SPLITS:
                            r0, r1 = OUT_SPLITS[yc]
                            q = list(OUT_SPLITS).index(yc)
                            eng = nc.sync if (img + cot + q) % 2 == 0 else nc.scalar
                            last = img == BL - 1 and cot == NCOT - 1 and yc == NYC - 1
                            if last:
                                # split the final store so the exit barrier
                                # waits on a small transfer
                                rm = (r0 + r1) // 2
                                nc.sync.dma_start(
                                    y_d[img, 128 * cot : 128 * (cot + 1), r0:rm, :],
                                    o_sb[:, r0:rm, :],
                                )
                                nc.scalar.dma_start(
                                    y_d[img, 128 * cot : 128 * (cot + 1), rm:r1, :],
                                    o_sb[:, rm:r1, :],
                                )
                            else:
                                eng.dma_start(
                                    y_d[img, 128 * cot : 128 * (cot + 1), r0:r1, :],
                                    o_sb[:, r0:r1, :],
                                )

    nc.compile()
    _cache["nc"] = nc
    return nc


def _in_maps(inputs, weight, bias):
    x = np.asarray(inputs).astype(MM_NP)
    # weight (co, ci, ky, kx) -> (ci, cot, tap, co_in_tile), taps in TAP_ORDER
    wt = (
        np.asarray(weight)
        .reshape(NCOT, 128, CIN, R, S)
        .transpose(2, 0, 3, 4, 1)  # (ci, cot, ky, kx, co)
        .astype(MM_NP)
    )
    w = np.ascontiguousarray(
        np.stack([wt[:, :, ky, kx, :] for ky, kx in TAP_ORDER], axis=2)
    )
    b = np.ascontiguousarray(
        np.asarray(bias).astype(np.float32).reshape(NCOT, 128).T
    )
    return [
        {"x": np.ascontiguousarray(x[c * BL : (c + 1) * BL]), "w": w, "b": b}
        for c in range(N_CORES)
    ]


def kernel(inputs, weight, bias):
    nc = _build()
    in_maps = _in_maps(inputs, weight, bias)
    res = run_bass_kernel_spmd(nc, in_maps, core_ids=list(range(N_CORES)))
    return np.concatenate([res.results[c]["y"] for c in range(N_CORES)], axis=0)



# revision 8
# speedup vs baseline: 1.0014x; 1.0014x over previous
"""Conv2d(128->256, 3x3, pad=1) over (32,128,56,56), data-parallel across 8
NeuronCores (4 images per core), with 1D Winograd F(2,3) along y.

Per core, per image: the host precomputes the y-direction Winograd input
butterfly U_j (j=0..3, each [Cin, 28ty, 56x] bf16) and the y-transformed
weights W'_j = (G w)_j.  The device then computes, for each output-row-pair
band (7 ty rows at a time):

  M_j[co, ty, x] = sum_kx sum_ci W'_j[ci, co, kx] * U_j[ci, ty, x+kx-1]

as 12 accumulating matmuls (4 j-planes x 3 x-taps, x-padding by clipping)
into 4 PSUM banks, then the inverse transform + bias on Vector/GpSimd:

  out[2ty]   = M_0 + M_1 + M_2 + b      (t0 = M0+M1 on DVE; Y0 = (M2+b)+t0)
  out[2ty+1] = M_1 - M_2 - M_3 + b      (t1 = M1-M2 on GpSimd; Y1 = (t1+b)-M3)

This is 6 MACs/output instead of 9 (direct 3x3), cutting PE time ~1/3.
Y rows interleave (stride-2 row writes) into a bf16 output tile which is
DMA'd back; the host casts bf16 -> f32 (tolerance 2e-2 >> bf16 noise).

Latency structure follows the proven baseline: first image's U loads in
ty-quarters on the Scalar HWDGE ring while weights load split across the
Sync/GpSimd rings; zero warmup matmuls ramp the PE p-state; images 1..3
load up-front on Sync; stores alternate Sync/Scalar, final store split
small so the exit barrier waits on a small transfer.
"""

import numpy as np
import ml_dtypes

import concourse.mybir as mybir
import concourse.tile as tile
from concourse import bacc
from concourse.bass_utils import run_bass_kernel_spmd

N_CORES = 8
B, CIN, H, W = 32, 128, 56, 56
COUT, R, S = 256, 3, 3
BL = B // N_CORES          # images per core
NCOT = COUT // 128         # Cout tiles of 128
NTY = H // 2               # 28 output row-pairs
TYC = 7                    # row-pairs per PSUM band
NCH = NTY // TYC           # 4 bands per (img, cot)

MM_DT = mybir.dt.bfloat16
MM_NP = ml_dtypes.bfloat16

NWARM = 6
ADD = mybir.AluOpType.add
SUB = mybir.AluOpType.subtract

_cache = {}


def _build():
    if "nc" in _cache:
        return _cache["nc"]
    nc = bacc.Bacc("TRN2", target_bir_lowering=False, debug=False)
    f32 = mybir.dt.float32
    u_d = nc.dram_tensor("u", [BL, CIN, 4, NTY, W], MM_DT, kind="ExternalInput").ap()
    w_d = nc.dram_tensor("w", [CIN, NCOT, 4, S, 128], MM_DT, kind="ExternalInput").ap()
    b_d = nc.dram_tensor("b", [128, NCOT], f32, kind="ExternalInput").ap()
    y_d = nc.dram_tensor("y", [BL, NCOT, 128, H, W], MM_DT, kind="ExternalOutput").ap()

    with tile.TileContext(nc) as tc:
        with (
            tc.tile_pool(name="consts", bufs=1) as cpool,
            tc.tile_pool(name="uin", bufs=BL) as upool,
            tc.tile_pool(name="yout", bufs=2) as opool,
            tc.tile_pool(name="s1s", bufs=2) as s1pool,
            tc.tile_pool(name="t0s", bufs=2) as t0pool,
            tc.tile_pool(name="t1s", bufs=2) as t1pool,
            tc.tile_pool(name="vs", bufs=2) as vpool,
            tc.tile_pool(name="ps", bufs=2, space="PSUM") as pspool,
        ):
            # --- PE prewarm: zero matmuls with no DMA dependency ---
            warm_x = cpool.tile([128, 512], MM_DT)
            nc.vector.memset(warm_x[:], 0.0)
            warm_ps = pspool.tile([128, 4, 8, 64], f32, tag="ps")
            for _ in range(NWARM):
                nc.tensor.matmul(
                    warm_ps[:, 0, :, :], warm_x[:, 0:128], warm_x[:],
                    start=True, stop=True,
                )

            # --- constants + first image, on parallel HWDGE rings ---
            w_sb = cpool.tile([CIN, NCOT, 4, S, 128], MM_DT)
            # cot-0 taps split across two rings so both halves land fast
            nc.sync.dma_start(w_sb[:, 0, 0:2], w_d[:, 0, 0:2])
            nc.gpsimd.dma_start(w_sb[:, 0, 2:4], w_d[:, 0, 2:4])
            nc.sync.dma_start(w_sb[:, 1], w_d[:, 1])
            b_sb = cpool.tile([128, NCOT], f32)
            nc.sync.dma_start(b_sb[:], b_d[:])

            # first image's U in ty-band quarters on the Scalar ring (matches
            # band consumption order); later images whole on Sync
            u_tiles = []
            u0 = upool.tile([CIN, 4, NTY, W], MM_DT, name="u_sb_0", tag="u_sb")
            for c in range(NCH):
                nc.scalar.dma_start(
                    u0[:, :, TYC * c : TYC * (c + 1), :],
                    u_d[0, :, :, TYC * c : TYC * (c + 1), :],
                )
            u_tiles.append(u0)
            for img in range(1, BL):
                u_sb = upool.tile([CIN, 4, NTY, W], MM_DT, name=f"u_sb_{img}", tag="u_sb")
                nc.sync.dma_start(u_sb[:], u_d[img])
                u_tiles.append(u_sb)

            nstore = 0
            for img in range(BL):
                u_sb = u_tiles[img]
                for cot in range(NCOT):
                    o_sb = opool.tile(
                        [128, H, W], MM_DT, name=f"o_sb_{img}_{cot}", tag="o_sb"
                    )
                    for ch in range(NCH):
                        ty0 = TYC * ch
                        ps = pspool.tile(
                            [128, 4, 8, 64], f32, name=f"ps_{img}_{cot}_{ch}", tag="ps"
                        )
                        for j in (1, 0, 2, 3):
                            # kx=1 covers the full band -> start=True first
                            for ki, kx in enumerate((1, 0, 2)):
                                ox0 = max(0, 1 - kx)
                                ox1 = min(W, W + 1 - kx)
                                nc.tensor.matmul(
                                    ps[:, j, 0:TYC, ox0:ox1],
                                    w_sb[:, cot, j, kx, :],
                                    u_sb[
                                        :, j, ty0 : ty0 + TYC,
                                        ox0 + kx - 1 : ox1 + kx - 1,
                                    ],
                                    start=(ki == 0),
                                    stop=(ki == 2),
                                )
                        # inverse transform + bias. GPSIMD cannot read PSUM,
                        # so each PSUM bank is read exactly once (3x DVE,
                        # 1x ACT) and GpSimd does the SBUF-only algebra:
                        #   s1 = M1+b   (ACT, PSUM)
                        #   t0 = M0+s1  (DVE, PSUM)
                        #   u  = s1-M2  (DVE, PSUM)   [= Y1 + M3]
                        #   Y1 = u-M3   (DVE, PSUM) -> odd rows
                        #   v  = t0-u   (Pool, SBUF)  [= M0+M2+... -> Y0-s1]
                        #   Y0 = v+s1   (Pool, SBUF) -> even rows
                        s1 = s1pool.tile([128, TYC, W], MM_DT, tag="s1")
                        t0 = t0pool.tile([128, TYC, W], MM_DT, tag="t0")
                        u = t1pool.tile([128, TYC, W], MM_DT, tag="u")
                        v = vpool.tile([128, TYC, W], MM_DT, tag="v")
                        nc.scalar.activation(
                            s1[:],
                            ps[:, 1, 0:TYC, 0:W],
                            mybir.ActivationFunctionType.Identity,
                            bias=b_sb[:, cot : cot + 1],
                        )
                        nc.vector.tensor_add(
                            t0[:], ps[:, 0, 0:TYC, 0:W], s1[:]
                        )
                        nc.vector.tensor_sub(
                            u[:], s1[:], ps[:, 2, 0:TYC, 0:W]
                        )
                        nc.vector.tensor_sub(
                            o_sb[:, 2 * ty0 + 1 : 2 * ty0 + 2 * TYC : 2, :],
                            u[:],
                            ps[:, 3, 0:TYC, 0:W],
                        )
                        nc.gpsimd.tensor_sub(v[:], t0[:], u[:])
                        nc.gpsimd.tensor_add(
                            o_sb[:, 2 * ty0 : 2 * ty0 + 2 * TYC : 2, :],
                            v[:],
                            s1[:],
                        )
                        # store the finished 14-row slab, alternating rings
                        r0, r1 = 2 * ty0, 2 * ty0 + 2 * TYC
                        eng = nc.sync if nstore % 2 == 0 else nc.scalar
                        nstore += 1
                        last = img == BL - 1 and cot == NCOT - 1 and ch == NCH - 1
                        if last:
                            rm = r1 - 4
                            nc.sync.dma_start(
                                y_d[img, cot, :, r0:rm, :], o_sb[:, r0:rm, :]
                            )
                            nc.scalar.dma_start(
                                y_d[img, cot, :, rm:r1, :], o_sb[:, rm:r1, :]
                            )
                        else:
                            eng.dma_start(
                                y_d[img, cot, :, r0:r1, :], o_sb[:, r0:r1, :]
                            )

    nc.compile()
    _cache["nc"] = nc
    return nc


def _in_maps(inputs, weight, bias):
    x = np.asarray(inputs, dtype=np.float32)
    # y-direction Winograd butterfly on the (row-padded) input
    xp = np.zeros((B, CIN, H + 2, W), np.float32)
    xp[:, :, 1 : H + 1] = x
    a0 = xp[:, :, 0:56:2]
    a1 = xp[:, :, 1:57:2]
    a2 = xp[:, :, 2:58:2]
    a3 = xp[:, :, 3:59:2]
    u = np.ascontiguousarray(
        np.stack([a0 - a2, a1 + a2, a2 - a1, a1 - a3], axis=2).astype(MM_NP)
    )  # [B, CIN, 4, 28, 56]

    # weights: W'_j = sum_ky G[j,ky] w[..,ky,..]; layout (ci, cot, j, kx, co)
    G = np.array(
        [[1, 0, 0], [0.5, 0.5, 0.5], [0.5, -0.5, 0.5], [0, 0, 1]], np.float32
    )
    wf = np.asarray(weight, dtype=np.float32)  # (co, ci, ky, kx)
    wj = np.einsum("jk,oiky->oijy", G, wf)     # (co, ci, j, kx)
    wt = (
        wj.reshape(NCOT, 128, CIN, 4, S)
        .transpose(2, 0, 3, 4, 1)               # (ci, cot, j, kx, co)
        .astype(MM_NP)
    )
    w = np.ascontiguousarray(wt)
    b = np.ascontiguousarray(
        np.asarray(bias).astype(np.float32).reshape(NCOT, 128).T
    )
    return [
        {"u": np.ascontiguousarray(u[c * BL : (c + 1) * BL]), "w": w, "b": b}
        for c in range(N_CORES)
    ]


def kernel(inputs, weight, bias):
    nc = _build()
    in_maps = _in_maps(inputs, weight, bias)
    res = run_bass_kernel_spmd(nc, in_maps, core_ids=list(range(N_CORES)))
    out = np.concatenate(
        [res.results[c]["y"] for c in range(N_CORES)], axis=0
    )  # [B, NCOT, 128, H, W] bf16
    return np.ascontiguousarray(out.reshape(B, COUT, H, W).astype(np.float32))


# revision 11
# speedup vs baseline: 1.1089x; 1.1074x over previous
"""Conv2d(128->256, 3x3, pad=1) over (32,128,56,56), data-parallel across 8
NeuronCores (4 images per core), with 1D Winograd F(2,3) along y.

Per core, per image: the host precomputes the y-direction Winograd input
butterfly U_j (j=0..3, each [Cin, 28ty, 56x] bf16) and the y-transformed
weights W'_j = (G w)_j.  The device then computes, for each output-row-pair
band (7 ty rows at a time):

  M_j[co, ty, x] = sum_kx sum_ci W'_j[ci, co, kx] * U_j[ci, ty, x+kx-1]

as 12 accumulating matmuls (4 j-planes x 3 x-taps, x-padding by clipping)
into 4 PSUM banks, then the inverse transform + bias on Vector/GpSimd:

  out[2ty]   = M_0 + M_1 + M_2 + b      (t0 = M0+M1 on DVE; Y0 = (M2+b)+t0)
  out[2ty+1] = M_1 - M_2 - M_3 + b      (t1 = M1-M2 on GpSimd; Y1 = (t1+b)-M3)

This is 6 MACs/output instead of 9 (direct 3x3), cutting PE time ~1/3.
Y rows interleave (stride-2 row writes) into a bf16 output tile which is
DMA'd back; the host casts bf16 -> f32 (tolerance 2e-2 >> bf16 noise).

Latency structure follows the proven baseline: first image's U loads in
ty-quarters on the Scalar HWDGE ring while weights load split across the
Sync/GpSimd rings; zero warmup matmuls ramp the PE p-state; images 1..3
load up-front on Sync; stores alternate Sync/Scalar, final store split
small so the exit barrier waits on a small transfer.
"""

import numpy as np
import ml_dtypes

import concourse.mybir as mybir
import concourse.tile as tile
from concourse import bacc
from concourse.bass_utils import run_bass_kernel_spmd

N_CORES = 8
B, CIN, H, W = 32, 128, 56, 56
COUT, R, S = 256, 3, 3
BL = B // N_CORES          # images per core
NCOT = COUT // 128         # Cout tiles of 128
NTY = H // 2               # 28 output row-pairs
TYC = 7                    # row-pairs per PSUM band
NCH = NTY // TYC           # 4 bands per (img, cot)

MM_DT = mybir.dt.bfloat16
MM_NP = ml_dtypes.bfloat16

NWARM = 6
ADD = mybir.AluOpType.add
SUB = mybir.AluOpType.subtract

_cache = {}


def _build():
    if "nc" in _cache:
        return _cache["nc"]
    nc = bacc.Bacc("TRN2", target_bir_lowering=False, debug=False)
    f32 = mybir.dt.float32
    u_d = nc.dram_tensor("u", [BL, CIN, 4, NTY, W], MM_DT, kind="ExternalInput").ap()
    w_d = nc.dram_tensor("w", [CIN, NCOT, 4, S, 128], MM_DT, kind="ExternalInput").ap()
    b_d = nc.dram_tensor("b", [128, NCOT], f32, kind="ExternalInput").ap()
    y_d = nc.dram_tensor("y", [BL, NCOT, 128, H, W], MM_DT, kind="ExternalOutput").ap()

    with tile.TileContext(nc) as tc:
        with (
            tc.tile_pool(name="consts", bufs=1) as cpool,
            tc.tile_pool(name="uin", bufs=BL) as upool,
            tc.tile_pool(name="yout", bufs=2) as opool,
            tc.tile_pool(name="s1s", bufs=2) as s1pool,
            tc.tile_pool(name="t0s", bufs=2) as t0pool,
            tc.tile_pool(name="t1s", bufs=2) as t1pool,
            tc.tile_pool(name="c2s", bufs=2) as c2pool,
            # one single-bank PSUM pool per Winograd j-plane: per-bank
            # release lets chunk n+2's matmuls start as soon as chunk n's
            # matching plane is drained (a single 4-bank tile would wait on
            # the whole chunk's LAST reader)
            tc.tile_pool(name="ps0", bufs=2, space="PSUM") as ps0pool,
            tc.tile_pool(name="ps1", bufs=2, space="PSUM") as ps1pool,
            tc.tile_pool(name="ps2", bufs=2, space="PSUM") as ps2pool,
            tc.tile_pool(name="ps3", bufs=2, space="PSUM") as ps3pool,
        ):
            jpool = {0: ps0pool, 1: ps1pool, 2: ps2pool, 3: ps3pool}
            # --- PE prewarm: zero matmuls with no DMA dependency ---
            warm_x = cpool.tile([128, 512], MM_DT)
            nc.vector.memset(warm_x[:], 0.0)
            warm_ps = ps3pool.tile([128, 8, 64], f32, tag="ps3")
            for _ in range(NWARM):
                nc.tensor.matmul(
                    warm_ps[:], warm_x[:, 0:128], warm_x[:],
                    start=True, stop=True,
                )

            # --- constants + first image, on parallel HWDGE rings ---
            w_sb = cpool.tile([CIN, NCOT, 4, S, 128], MM_DT)
            # cot-0 taps split across two rings so both halves land fast
            nc.sync.dma_start(w_sb[:, 0, 0:2], w_d[:, 0, 0:2])
            nc.gpsimd.dma_start(w_sb[:, 0, 2:4], w_d[:, 0, 2:4])
            nc.sync.dma_start(w_sb[:, 1], w_d[:, 1])
            b_sb = cpool.tile([128, NCOT], f32)
            nc.sync.dma_start(b_sb[:], b_d[:])

            # first image's U in ty-band quarters on the Scalar ring (matches
            # band consumption order); later images whole on Sync
            u_tiles = []
            u0 = upool.tile([CIN, 4, NTY, W], MM_DT, name="u_sb_0", tag="u_sb")
            for c in range(NCH):
                nc.scalar.dma_start(
                    u0[:, :, TYC * c : TYC * (c + 1), :],
                    u_d[0, :, :, TYC * c : TYC * (c + 1), :],
                )
            u_tiles.append(u0)
            # images 1..3 on the GpSimd SWDGE queue so the Sync/Scalar HWDGE
            # rings stay free for output stores (a 1.6MB image load on the
            # store ring stalls slab stores -> o_sb slots -> the PE)
            for img in range(1, BL):
                u_sb = upool.tile([CIN, 4, NTY, W], MM_DT, name=f"u_sb_{img}", tag="u_sb")
                nc.gpsimd.dma_start(u_sb[:], u_d[img])
                u_tiles.append(u_sb)

            nstore = 0
            for img in range(BL):
                u_sb = u_tiles[img]
                for cot in range(NCOT):
                    o_sb = opool.tile(
                        [128, H, W], MM_DT, name=f"o_sb_{img}_{cot}", tag="o_sb"
                    )
                    for ch in range(NCH):
                        ty0 = TYC * ch
                        ps = {
                            j: jpool[j].tile(
                                [128, 8, 64], f32,
                                name=f"ps{j}_{img}_{cot}_{ch}", tag=f"ps{j}",
                            )
                            for j in range(4)
                        }
                        for j in (1, 2, 0, 3):
                            # kx=1 covers the full band -> start=True first
                            for ki, kx in enumerate((1, 0, 2)):
                                ox0 = max(0, 1 - kx)
                                ox1 = min(W, W + 1 - kx)
                                nc.tensor.matmul(
                                    ps[j][:, 0:TYC, ox0:ox1],
                                    w_sb[:, cot, j, kx, :],
                                    u_sb[
                                        :, j, ty0 : ty0 + TYC,
                                        ox0 + kx - 1 : ox1 + kx - 1,
                                    ],
                                    start=(ki == 0),
                                    stop=(ki == 2),
                                )
                        # inverse transform + bias. GPSIMD cannot read PSUM;
                        # each PSUM plane is read exactly once (2x ACT, 2x
                        # DVE), the rest is SBUF-only bf16:
                        #   s1 = M1+b   (ACT, PSUM)
                        #   c2 = M2     (ACT Copy, PSUM)
                        #   t0 = M0+s1  (DVE, PSUM)
                        #   t1 = s1-c2  (Pool, SBUF)
                        #   Y0 = t0+c2  (DVE, SBUF) -> even rows
                        #   Y1 = t1-M3  (DVE, PSUM) -> odd rows
                        s1 = s1pool.tile([128, TYC, W], MM_DT, tag="s1")
                        c2 = c2pool.tile([128, TYC, W], MM_DT, tag="c2")
                        t0 = t0pool.tile([128, TYC, W], MM_DT, tag="t0")
                        t1 = t1pool.tile([128, TYC, W], MM_DT, tag="t1")
                        nc.scalar.activation(
                            s1[:],
                            ps[1][:, 0:TYC, 0:W],
                            mybir.ActivationFunctionType.Identity,
                            bias=b_sb[:, cot : cot + 1],
                        )
                        nc.scalar.activation(
                            c2[:],
                            ps[2][:, 0:TYC, 0:W],
                            mybir.ActivationFunctionType.Copy,
                        )
                        nc.vector.tensor_add(
                            t0[:], ps[0][:, 0:TYC, 0:W], s1[:]
                        )
                        nc.gpsimd.tensor_sub(t1[:], s1[:], c2[:])
                        nc.vector.tensor_add(
                            o_sb[:, 2 * ty0 : 2 * ty0 + 2 * TYC : 2, :],
                            t0[:],
                            c2[:],
                        )
                        nc.vector.tensor_sub(
                            o_sb[:, 2 * ty0 + 1 : 2 * ty0 + 2 * TYC : 2, :],
                            t1[:],
                            ps[3][:, 0:TYC, 0:W],
                        )
                        # store the finished 14-row slab, alternating rings
                        r0, r1 = 2 * ty0, 2 * ty0 + 2 * TYC
                        eng = nc.sync if nstore % 2 == 0 else nc.scalar
                        nstore += 1
                        last = img == BL - 1 and cot == NCOT - 1 and ch == NCH - 1
                        if last:
                            rm = r1 - 4
                            nc.sync.dma_start(
                                y_d[img, cot, :, r0:rm, :], o_sb[:, r0:rm, :]
                            )
                            nc.scalar.dma_start(
                                y_d[img, cot, :, rm:r1, :], o_sb[:, rm:r1, :]
                            )
                        else:
                            eng.dma_start(
                                y_d[img, cot, :, r0:r1, :], o_sb[:, r0:r1, :]
                            )

    nc.compile()
    _cache["nc"] = nc
    return nc


def _in_maps(inputs, weight, bias):
    x = np.asarray(inputs, dtype=np.float32)
    # y-direction Winograd butterfly on the (row-padded) input
    xp = np.zeros((B, CIN, H + 2, W), np.float32)
    xp[:, :, 1 : H + 1] = x
    a0 = xp[:, :, 0:56:2]
    a1 = xp[:, :, 1:57:2]
    a2 = xp[:, :, 2:58:2]
    a3 = xp[:, :, 3:59:2]
    u = np.ascontiguousarray(
        np.stack([a0 - a2, a1 + a2, a2 - a1, a1 - a3], axis=2).astype(MM_NP)
    )  # [B, CIN, 4, 28, 56]

    # weights: W'_j = sum_ky G[j,ky] w[..,ky,..]; layout (ci, cot, j, kx, co)
    G = np.array(
        [[1, 0, 0], [0.5, 0.5, 0.5], [0.5, -0.5, 0.5], [0, 0, 1]], np.float32
    )
    wf = np.asarray(weight, dtype=np.float32)  # (co, ci, ky, kx)
    wj = np.einsum("jk,oiky->oijy", G, wf)     # (co, ci, j, kx)
    wt = (
        wj.reshape(NCOT, 128, CIN, 4, S)
        .transpose(2, 0, 3, 4, 1)               # (ci, cot, j, kx, co)
        .astype(MM_NP)
    )
    w = np.ascontiguousarray(wt)
    b = np.ascontiguousarray(
        np.asarray(bias).astype(np.float32).reshape(NCOT, 128).T
    )
    return [
        {"u": np.ascontiguousarray(u[c * BL : (c + 1) * BL]), "w": w, "b": b}
        for c in range(N_CORES)
    ]


def kernel(inputs, weight, bias):
    nc = _build()
    in_maps = _in_maps(inputs, weight, bias)
    res = run_bass_kernel_spmd(nc, in_maps, core_ids=list(range(N_CORES)))
    out = np.concatenate(
        [res.results[c]["y"] for c in range(N_CORES)], axis=0
    )  # [B, NCOT, 128, H, W] bf16
    return np.ascontiguousarray(out.reshape(B, COUT, H, W).astype(np.float32))
